# revision 27
# baseline (speedup 1.0000x reference)
"""FSUMGU cell on 8 Trainium2 NeuronCores — v2.

Math (per reference):
    zf = [hx, x] @ w_f.T + b_f
    fg = (zf + 1) / 2
    fgx = fg * hx
    ng = [fgx, x] @ w_n.T + b_n
    hy = (1 - fg) * ng + fgx

Sharding: 4 batch groups (512 rows) x 2 hidden halves (1024 cols);
core = 2*g + hc.  Pair {2g, 2g+1} shares batch rows, splits hidden.

Formulation: transposed-output [h, b].  Weights are the stationary
matmul operand (lhsT [k,128h]); activations stream as rhs [k, 512b].
All tensors arrive from the host already bf16, transposed, and laid
out partition-major, so the device issues NO PE transposes, NO bias
broadcasts, NO cast traffic: the tensor engine runs only the 512 real
matmuls per core.  Biases are per-partition columns consumed by
scalar-engine Identity activations (out = in*scale + bias).

hx^T k-tiles are ordered my-slice-first (w_f rows reordered to match)
so fgx = fg * hxT[:, a, :] needs no per-core slice of hx.  The fgx
exchange is a pair AllGather done in two chunks (local h-tiles 0-3,
4-7) that ride under GEMM2's input-half contraction; GEMM2 holds all
8 PSUM banks and contracts x-half first, then gathered-fgx halves as
the chunks land.
"""
import sys

sys.path.insert(0, "/opt/trn_rl_repo")

import numpy as np
import ml_dtypes
import concourse.bass as bass
import concourse.tile as tile
from concourse import bacc, mybir
from concourse.bass_utils import run_bass_kernel_spmd

F32 = mybir.dt.float32
BF16 = mybir.dt.bfloat16
BF = ml_dtypes.bfloat16
IDENT = mybir.ActivationFunctionType.Identity

B, H, I = 2048, 2048, 2048
R, C = 4, 2            # batch groups x hidden halves
BL = B // R            # 512 batch rows per core
HC = H // C            # 1024 hidden outputs per core
NHT = HC // 128        # 8 local h-tiles
NKT = 32               # 4096 contraction / 128

_NC_CACHE = None


def build():
    nc = bacc.Bacc(None, target_bir_lowering=False, debug=False)
    # p-major layouts: partition dim first, >=4KB contiguous per partition
    d_hxT = nc.dram_tensor("hxT", [128, 16, BL], BF16, kind="ExternalInput").ap()
    d_inpT = nc.dram_tensor("inpT", [128, 16, BL], BF16, kind="ExternalInput").ap()
    d_wf = nc.dram_tensor("wf", [128, 16, 16, 128], BF16, kind="ExternalInput").ap()
    d_wn = nc.dram_tensor("wn", [128, NHT, NKT, 128], BF16, kind="ExternalInput").ap()
    d_bfp = nc.dram_tensor("bfp", [128, NHT], F32, kind="ExternalInput").ap()
    d_bfm = nc.dram_tensor("bfm", [128, NHT], F32, kind="ExternalInput").ap()
    d_bn = nc.dram_tensor("bn", [128, NHT], F32, kind="ExternalInput").ap()
    d_hy = nc.dram_tensor("hy", [NHT, 128, BL], F32, kind="ExternalOutput").ap()

    with tile.TileContext(nc) as tc:
        with (
            tc.tile_pool(name="const", bufs=1) as const,
            tc.tile_pool(name="wts", bufs=1) as wts,
            tc.tile_pool(name="acts", bufs=1) as acts,      # hxT (then gat), inpT
            tc.tile_pool(name="persist", bufs=1) as persist,
            tc.tile_pool(name="fg_sc", bufs=2) as fg_sc,
            tc.tile_pool(name="ng_sc", bufs=2) as ng_sc,
            tc.tile_pool(name="hy_sc", bufs=2) as hy_sc,
            tc.tile_pool(name="dram", bufs=1, space="DRAM") as dram,
            tc.tile_pool(name="ps", bufs=8, space="PSUM") as ps,
        ):
            # ---- load plan.  Every dma_start costs ~0.6us of issuing-engine
            # time and stalls on queue-full, so each engine gets one role:
            #   sync:   biases, hxT, wf hi-chunks, wn x-half, gat, in order
            #   scalar: 8 wf lo-chunk issues up front, then ONLY activations
            #   gpsimd: inpT, then the CC chain (+ wn fgx-half), then hy
            # Ring FIFO order == consumption order; hxT/inpT ride parallel
            # rings so GEMM1 h-tile 0 is paced by ~2MB, not 4MB.
            wf = wts.tile([128, 16, 16, 128], BF16, tag="wf")
            wn = wts.tile([128, NHT, NKT, 128], BF16, tag="wn")
            hxT = acts.tile([128, 16, BL], BF16, tag="hx_gat")
            inpT = acts.tile([128, 16, BL], BF16, tag="inp")
            bfp = const.tile([128, NHT], F32, tag="bfp")
            bfm = const.tile([128, NHT], F32, tag="bfm")
            bn = const.tile([128, NHT], F32, tag="bn")

            # gpsimd is the software-DGE path (slow descriptor gen) — bulk
            # loads go only through the two HWDGE rings (sync + scalar),
            # FIFO-ordered by first consumption time: sync paces GEMM1's
            # streamed activations, scalar delivers the stationary weights.
            # sync paces GEMM1's streamed activations (leading edge split
            # fine so the first matmuls start early); scalar delivers the
            # stationary weights in consumption order.
            nc.sync.dma_start(hxT[:, 0:2], d_hxT[:, 0:2])
            nc.sync.dma_start(hxT[:, 2:4], d_hxT[:, 2:4])
            for t in range(4, 16, 4):
                nc.sync.dma_start(hxT[:, t:t + 4], d_hxT[:, t:t + 4])
            for t in range(0, 16, 4):
                nc.sync.dma_start(inpT[:, t:t + 4], d_inpT[:, t:t + 4])
            for a in range(NHT):    # fgx-half weights, needed at pass 2a;
                nc.sync.dma_start(wn[:, a, 0:16], d_wn[:, a, 0:16])

            nc.scalar.dma_start(wf[:, 0, 0:8], d_wf[:, 0, 0:8])
            nc.scalar.dma_start(wf[:, 0, 8:16], d_wf[:, 0, 8:16])
            nc.scalar.dma_start(wf[:, 1], d_wf[:, 1])
            nc.scalar.dma_start(bfp[:], d_bfp[:])
            nc.scalar.dma_start(bfm[:], d_bfm[:])
            nc.scalar.dma_start(bn[:], d_bn[:])
            for c in range(2, 16):
                nc.scalar.dma_start(wf[:, c], d_wf[:, c])
            for a in range(NHT):    # x-half weights, needed at pass 1
                nc.scalar.dma_start(wn[:, a, 16:32], d_wn[:, a, 16:32])

            # fgx split per CC chunk so each bounce write's dependency
            # closes as soon as its 4 h-tiles are done
            fgx1 = persist.tile([128, 4, BL], BF16, tag="fgx1")
            fgx2 = persist.tile([128, 4, BL], BF16, tag="fgx2")
            omfg = persist.tile([128, NHT, BL], BF16, tag="omfg")

            cc_in1 = dram.tile([128, 4, BL], BF16)
            cc_in2 = dram.tile([128, 4, BL], BF16)
            cc_out1 = dram.tile([2, 128, 4, BL], BF16)
            cc_out2 = dram.tile([2, 128, 4, BL], BF16)

            # ---- GEMM1: zf^T per h-tile; drain to fgx / omfg
            for a in range(NHT):
                acc = ps.tile([128, BL], F32, tag="acc")
                for j in range(NKT):
                    lhsT = wf[:, a * 2 + j // 16, j % 16, :]
                    rhs = hxT[:, j, :] if j < 16 else inpT[:, j - 16, :]
                    nc.tensor.matmul(acc[:], lhsT, rhs,
                                     start=(j == 0), stop=(j == NKT - 1))
                fg_t = fg_sc.tile([128, BL], BF16, tag="fg")
                nc.scalar.activation(fg_t[:], acc[:], IDENT,
                                     bias=bfp[:, a:a + 1], scale=0.5)
                nc.scalar.activation(omfg[:, a, :], acc[:], IDENT,
                                     bias=bfm[:, a:a + 1], scale=-0.5)
                fgx_t = fgx1 if a < 4 else fgx2
                nc.vector.tensor_mul(fgx_t[:, a % 4, :], fg_t[:], hxT[:, a, :])

            # ---- pair AllGather of fgx^T, two chunks (gpsimd stream:
            # bounce write -> trigger, per chunk; waits ride on gpsimd)
            nc.gpsimd.dma_start(cc_in1[:], fgx1[:])
            nc.gpsimd.collective_compute(
                "AllGather", mybir.AluOpType.bypass,
                replica_groups=[[0, 1], [2, 3], [4, 5], [6, 7]],
                ins=[cc_in1.opt()], outs=[cc_out1.opt()],
            )
            nc.gpsimd.dma_start(cc_in2[:], fgx2[:])
            nc.gpsimd.collective_compute(
                "AllGather", mybir.AluOpType.bypass,
                replica_groups=[[0, 1], [2, 3], [4, 5], [6, 7]],
                ins=[cc_in2.opt()], outs=[cc_out2.opt()],
            )

            # gathered fgx^T in global k order, reusing hxT's SBUF slot
            gat = acts.tile([128, 16, BL], BF16, tag="hx_gat")
            for m in range(2):
                nc.sync.dma_start(gat[:, m * 8:m * 8 + 4], cc_out1[m])
            for m in range(2):
                nc.sync.dma_start(gat[:, m * 8 + 4:m * 8 + 8], cc_out2[m])

            # ---- GEMM2: x-half first (CC-independent), all 8 banks held;
            # then gathered-fgx halves as the chunks land.
            acc2 = []
            for a in range(NHT):
                t = ps.tile([128, BL], F32, tag="acc")
                acc2.append(t)
                for j in range(16, 32):
                    nc.tensor.matmul(t[:], wn[:, a, j, :], inpT[:, j - 16, :],
                                     start=(j == 16), stop=False)
            for a in range(NHT):
                for j in (0, 1, 2, 3, 8, 9, 10, 11):
                    nc.tensor.matmul(acc2[a][:], wn[:, a, j, :], gat[:, j, :],
                                     start=False, stop=False)
            for a in range(NHT):
                for i, j in enumerate((4, 5, 6, 7, 12, 13, 14, 15)):
                    nc.tensor.matmul(acc2[a][:], wn[:, a, j, :], gat[:, j, :],
                                     start=False, stop=(i == 7))
                ng_t = ng_sc.tile([128, BL], F32, tag="ng")
                nc.scalar.activation(ng_t[:], acc2[a][:], IDENT,
                                     bias=bn[:, a:a + 1], scale=1.0)
                fgx_t = fgx1 if a < 4 else fgx2
                hy_t = hy_sc.tile([128, BL], F32, tag="hy")
                nc.vector.tensor_mul(hy_t[:], omfg[:, a, :], ng_t[:])
                nc.vector.tensor_add(hy_t[:], hy_t[:], fgx_t[:, a % 4, :])
                nc.sync.dma_start(d_hy[a], hy_t[:])  # HWDGE, no cast

    nc.finalize()
    return nc


def _get_nc():
    global _NC_CACHE
    if _NC_CACHE is None:
        _NC_CACHE = build()
    return _NC_CACHE


def make_in_maps(input, hx, w_f, b_f, w_n, b_n):
    """Host-side shard + transpose + bf16 cast into p-major device layouts."""
    input = np.asarray(input, dtype=np.float32)
    hx = np.asarray(hx, dtype=np.float32)
    w_f = np.asarray(w_f, dtype=np.float32)
    b_f = np.asarray(b_f, dtype=np.float32)
    w_n = np.asarray(w_n, dtype=np.float32)
    b_n = np.asarray(b_n, dtype=np.float32)

    in_maps = []
    for core in range(R * C):
        g, hc = core // C, core % C
        rows = slice(g * BL, (g + 1) * BL)
        hsl = slice(hc * HC, (hc + 1) * HC)
        psl = slice((1 - hc) * HC, (2 - hc) * HC)

        # hx^T with k-tiles my-slice-first; [2048,512]->[128p,16kt,512b]
        hxTf = hx[rows].T
        hxTr = np.concatenate([hxTf[hsl], hxTf[psl]], axis=0)
        hxT = np.ascontiguousarray(
            hxTr.reshape(16, 128, BL).transpose(1, 0, 2).astype(BF))
        inpT = np.ascontiguousarray(
            input[rows].T.reshape(16, 128, BL).transpose(1, 0, 2).astype(BF))

        # w_f rows for my h-slice, k reordered to match hxT; chunk layout
        # [p, c=a*2+half, jj, q] with lhsT(a,j)[p,q] = W[a*128+q, j*128+p]
        Wf = w_f[hsl]
        Wfr = np.concatenate([Wf[:, hsl], Wf[:, psl], Wf[:, H:]], axis=1)
        wf = np.ascontiguousarray(
            Wfr.reshape(NHT, 128, NKT, 128).transpose(3, 0, 2, 1)
            .reshape(128, 16, 16, 128).astype(BF))
        # w_n natural k order (fgx domain is global), [p, a, j, q]
        wn = np.ascontiguousarray(
            w_n[hsl].reshape(NHT, 128, NKT, 128).transpose(3, 0, 2, 1)
            .astype(BF))

        bfp = (b_f[hsl] + 1.0) * 0.5
        in_maps.append({
            "hxT": hxT,
            "inpT": inpT,
            "wf": wf,
            "wn": wn,
            "bfp": np.ascontiguousarray(bfp.reshape(NHT, 128).T, dtype=np.float32),
            "bfm": np.ascontiguousarray((1.0 - bfp).reshape(NHT, 128).T, dtype=np.float32),
            "bn": np.ascontiguousarray(b_n[hsl].reshape(NHT, 128).T, dtype=np.float32),
        })
    return in_maps


def assemble(results):
    """[NHT,128,BL] bf16 hy^T per core -> full [B, H] f32."""
    out = np.empty((B, H), dtype=np.float32)
    for core in range(R * C):
        g, hc = core // C, core % C
        hyT = np.asarray(results[core]["hy"], dtype=np.float32)
        out[g * BL:(g + 1) * BL, hc * HC:(hc + 1) * HC] = \
            hyT.reshape(HC, BL).T
    return out


def kernel(input, hx, w_f, b_f, w_n, b_n, **_ignored):
    nc = _get_nc()
    in_maps = make_in_maps(input, hx, w_f, b_f, w_n, b_n)
    res = run_bass_kernel_spmd(nc, in_maps, list(range(R * C)))
    return assemble(res.results)


if __name__ == "__main__":
    rng = np.random.default_rng(0)
    inputs = {
        "input": rng.uniform(-1, 1, (B, I)).astype(np.float32),
        "hx": rng.uniform(-1, 1, (B, H)).astype(np.float32),
        "w_f": (rng.standard_normal((H, H + I)) / np.sqrt(H + I)).astype(np.float32),
        "b_f": (rng.standard_normal(H) / np.sqrt(H + I)).astype(np.float32),
        "w_n": (rng.standard_normal((H, H + I)) / np.sqrt(H + I)).astype(np.float32),
        "b_n": (rng.standard_normal(H) / np.sqrt(H + I)).astype(np.float32),
    }
    out = kernel(**inputs)
    x64 = {k: v.astype(np.float64) for k, v in inputs.items()}
    cat = np.concatenate([x64["hx"], x64["input"]], axis=1)
    fg = (cat @ x64["w_f"].T + x64["b_f"] + 1.0) * 0.5
    fgx = fg * x64["hx"]
    ng = np.concatenate([fgx, x64["input"]], axis=1) @ x64["w_n"].T + x64["b_n"]
    exp = (1.0 - fg) * ng + fgx
    err = np.abs(out - exp).max() / np.abs(exp).max()
    print("rel err:", err)


# revision 29
# speedup vs baseline: 1.1605x; 1.1605x over previous
"""FSUMGU cell on 8 Trainium2 NeuronCores — v2.

Math (per reference):
    zf = [hx, x] @ w_f.T + b_f
    fg = (zf + 1) / 2
    fgx = fg * hx
    ng = [fgx, x] @ w_n.T + b_n
    hy = (1 - fg) * ng + fgx

Sharding: 4 batch groups (512 rows) x 2 hidden halves (1024 cols);
core = 2*g + hc.  Pair {2g, 2g+1} shares batch rows, splits hidden.

Formulation: transposed-output [h, b].  Weights are the stationary
matmul operand (lhsT [k,128h]); activations stream as rhs [k, 512b].
All tensors arrive from the host already bf16, transposed, and laid
out partition-major, so the device issues NO PE transposes, NO bias
broadcasts, NO cast traffic: the tensor engine runs only the 512 real
matmuls per core.  Biases are per-partition columns consumed by
scalar-engine Identity activations (out = in*scale + bias).

hx^T k-tiles are ordered my-slice-first (w_f rows reordered to match)
so fgx = fg * hxT[:, a, :] needs no per-core slice of hx.  The fgx
exchange is a pair AllGather done in two chunks (local h-tiles 0-3,
4-7) that ride under GEMM2's input-half contraction; GEMM2 holds all
8 PSUM banks and contracts x-half first, then gathered-fgx halves as
the chunks land.
"""
import sys

sys.path.insert(0, "/opt/trn_rl_repo")

import numpy as np
import ml_dtypes
import concourse.bass as bass
import concourse.tile as tile
from concourse import bacc, mybir
from concourse.bass_utils import run_bass_kernel_spmd

F32 = mybir.dt.float32
BF16 = mybir.dt.bfloat16
BF = ml_dtypes.bfloat16
IDENT = mybir.ActivationFunctionType.Identity

B, H, I = 2048, 2048, 2048
R, C = 4, 2            # batch groups x hidden halves
BL = B // R            # 512 batch rows per core
HC = H // C            # 1024 hidden outputs per core
NHT = HC // 128        # 8 local h-tiles
NKT = 32               # 4096 contraction / 128

_NC_CACHE = None


def build():
    nc = bacc.Bacc(None, target_bir_lowering=False, debug=False)
    # p-major layouts: partition dim first, >=4KB contiguous per partition
    d_hxT = nc.dram_tensor("hxT", [128, 16, BL], BF16, kind="ExternalInput").ap()
    d_inpT = nc.dram_tensor("inpT", [128, 16, BL], BF16, kind="ExternalInput").ap()
    d_wf = nc.dram_tensor("wf", [128, 16, 16, 128], BF16, kind="ExternalInput").ap()
    d_wn = nc.dram_tensor("wn", [128, NHT, NKT, 128], BF16, kind="ExternalInput").ap()
    d_bfp = nc.dram_tensor("bfp", [128, NHT], F32, kind="ExternalInput").ap()
    d_bfm = nc.dram_tensor("bfm", [128, NHT], F32, kind="ExternalInput").ap()
    d_bn = nc.dram_tensor("bn", [128, NHT], F32, kind="ExternalInput").ap()
    d_hy = nc.dram_tensor("hy", [NHT, 128, BL], F32, kind="ExternalOutput").ap()

    with tile.TileContext(nc) as tc:
        with (
            tc.tile_pool(name="const", bufs=1) as const,
            tc.tile_pool(name="wts", bufs=1) as wts,
            tc.tile_pool(name="acts", bufs=1) as acts,      # hxT (then gat), inpT
            tc.tile_pool(name="persist", bufs=1) as persist,
            tc.tile_pool(name="fg_sc", bufs=2) as fg_sc,
            tc.tile_pool(name="ng_sc", bufs=2) as ng_sc,
            tc.tile_pool(name="hy_sc", bufs=2) as hy_sc,
            tc.tile_pool(name="dram", bufs=1, space="DRAM") as dram,
            tc.tile_pool(name="ps", bufs=8, space="PSUM") as ps,
        ):
            # ---- load plan.  Every dma_start costs ~0.6us of issuing-engine
            # time and stalls on queue-full, so each engine gets one role:
            #   sync:   biases, hxT, wf hi-chunks, wn x-half, gat, in order
            #   scalar: 8 wf lo-chunk issues up front, then ONLY activations
            #   gpsimd: inpT, then the CC chain (+ wn fgx-half), then hy
            # Ring FIFO order == consumption order; hxT/inpT ride parallel
            # rings so GEMM1 h-tile 0 is paced by ~2MB, not 4MB.
            wf = wts.tile([128, 16, 16, 128], BF16, tag="wf")
            wn = wts.tile([128, NHT, NKT, 128], BF16, tag="wn")
            hxT = acts.tile([128, 16, BL], BF16, tag="hx_gat")
            inpT = acts.tile([128, 16, BL], BF16, tag="inp")
            bfp = const.tile([128, NHT], F32, tag="bfp")
            bfm = const.tile([128, NHT], F32, tag="bfm")
            bn = const.tile([128, NHT], F32, tag="bn")

            # gpsimd is the software-DGE path (slow descriptor gen) — bulk
            # loads go only through the two HWDGE rings (sync + scalar),
            # FIFO-ordered by first consumption time: sync paces GEMM1's
            # streamed activations, scalar delivers the stationary weights.
            # sync paces GEMM1's streamed activations (leading edge split
            # fine so the first matmuls start early); scalar delivers the
            # stationary weights in consumption order.
            nc.sync.dma_start(hxT[:, 0:2], d_hxT[:, 0:2])
            nc.sync.dma_start(hxT[:, 2:4], d_hxT[:, 2:4])
            for t in range(4, 16, 4):
                nc.sync.dma_start(hxT[:, t:t + 4], d_hxT[:, t:t + 4])
            for t in range(0, 16, 4):
                nc.sync.dma_start(inpT[:, t:t + 4], d_inpT[:, t:t + 4])

            nc.scalar.dma_start(wf[:, 0, 0:8], d_wf[:, 0, 0:8])
            nc.scalar.dma_start(wf[:, 0, 8:16], d_wf[:, 0, 8:16])
            nc.scalar.dma_start(wf[:, 1], d_wf[:, 1])
            nc.scalar.dma_start(bfp[:], d_bfp[:])
            nc.scalar.dma_start(bfm[:], d_bfm[:])
            nc.scalar.dma_start(bn[:], d_bn[:])
            for c in range(2, 16):
                nc.scalar.dma_start(wf[:, c], d_wf[:, c])
            for a in range(NHT):    # x-half weights, needed at pass 1
                nc.scalar.dma_start(wn[:, a, 16:32], d_wn[:, a, 16:32])
            for a in range(NHT):    # fgx-half weights, needed at pass 2a
                nc.scalar.dma_start(wn[:, a, 0:16], d_wn[:, a, 0:16])

            # fgx split per CC chunk so each bounce write's dependency
            # closes as soon as its 4 h-tiles are done
            fgx1 = persist.tile([128, 4, BL], BF16, tag="fgx1")
            fgx2 = persist.tile([128, 4, BL], BF16, tag="fgx2")
            omfg = persist.tile([128, NHT, BL], BF16, tag="omfg")

            cc_in1 = dram.tile([128, 4, BL], BF16)
            cc_in2 = dram.tile([128, 4, BL], BF16)
            cc_out1 = dram.tile([2, 128, 4, BL], BF16)
            cc_out2 = dram.tile([2, 128, 4, BL], BF16)

            # ---- GEMM1: zf^T per h-tile; drain to fgx / omfg
            for a in range(NHT):
                acc = ps.tile([128, BL], F32, tag="acc")
                for j in range(NKT):
                    lhsT = wf[:, a * 2 + j // 16, j % 16, :]
                    rhs = hxT[:, j, :] if j < 16 else inpT[:, j - 16, :]
                    nc.tensor.matmul(acc[:], lhsT, rhs,
                                     start=(j == 0), stop=(j == NKT - 1))
                fg_t = fg_sc.tile([128, BL], BF16, tag="fg")
                nc.scalar.activation(fg_t[:], acc[:], IDENT,
                                     bias=bfp[:, a:a + 1], scale=0.5)
                nc.scalar.activation(omfg[:, a, :], acc[:], IDENT,
                                     bias=bfm[:, a:a + 1], scale=-0.5)
                fgx_t = fgx1 if a < 4 else fgx2
                nc.vector.tensor_mul(fgx_t[:, a % 4, :], fg_t[:], hxT[:, a, :])

            # ---- pair AllGather of fgx^T, two chunks (gpsimd stream:
            # bounce write -> trigger, per chunk; waits ride on gpsimd)
            nc.gpsimd.dma_start(cc_in1[:], fgx1[:])
            nc.gpsimd.collective_compute(
                "AllGather", mybir.AluOpType.bypass,
                replica_groups=[[0, 1], [2, 3], [4, 5], [6, 7]],
                ins=[cc_in1.opt()], outs=[cc_out1.opt()],
            )
            nc.gpsimd.dma_start(cc_in2[:], fgx2[:])
            nc.gpsimd.collective_compute(
                "AllGather", mybir.AluOpType.bypass,
                replica_groups=[[0, 1], [2, 3], [4, 5], [6, 7]],
                ins=[cc_in2.opt()], outs=[cc_out2.opt()],
            )

            # gathered fgx^T in global k order, reusing hxT's SBUF slot
            gat = acts.tile([128, 16, BL], BF16, tag="hx_gat")
            for m in range(2):
                nc.sync.dma_start(gat[:, m * 8:m * 8 + 4], cc_out1[m])
            for m in range(2):
                nc.sync.dma_start(gat[:, m * 8 + 4:m * 8 + 8], cc_out2[m])

            # ---- GEMM2: x-half first (CC-independent), all 8 banks held;
            # then gathered-fgx halves as the chunks land.
            acc2 = []
            for a in range(NHT):
                t = ps.tile([128, BL], F32, tag="acc")
                acc2.append(t)
                for j in range(16, 32):
                    nc.tensor.matmul(t[:], wn[:, a, j, :], inpT[:, j - 16, :],
                                     start=(j == 16), stop=False)
            for a in range(NHT):
                for j in (0, 1, 2, 3, 8, 9, 10, 11):
                    nc.tensor.matmul(acc2[a][:], wn[:, a, j, :], gat[:, j, :],
                                     start=False, stop=False)
            for a in range(NHT):
                for i, j in enumerate((4, 5, 6, 7, 12, 13, 14, 15)):
                    nc.tensor.matmul(acc2[a][:], wn[:, a, j, :], gat[:, j, :],
                                     start=False, stop=(i == 7))
                ng_t = ng_sc.tile([128, BL], F32, tag="ng")
                nc.scalar.activation(ng_t[:], acc2[a][:], IDENT,
                                     bias=bn[:, a:a + 1], scale=1.0)
                fgx_t = fgx1 if a < 4 else fgx2
                hy_t = hy_sc.tile([128, BL], F32, tag="hy")
                nc.vector.tensor_mul(hy_t[:], omfg[:, a, :], ng_t[:])
                nc.vector.tensor_add(hy_t[:], hy_t[:], fgx_t[:, a % 4, :])
                nc.sync.dma_start(d_hy[a], hy_t[:])  # HWDGE, no cast

    nc.finalize()
    return nc


def _get_nc():
    global _NC_CACHE
    if _NC_CACHE is None:
        _NC_CACHE = build()
    return _NC_CACHE


def make_in_maps(input, hx, w_f, b_f, w_n, b_n):
    """Host-side shard + transpose + bf16 cast into p-major device layouts."""
    input = np.asarray(input, dtype=np.float32)
    hx = np.asarray(hx, dtype=np.float32)
    w_f = np.asarray(w_f, dtype=np.float32)
    b_f = np.asarray(b_f, dtype=np.float32)
    w_n = np.asarray(w_n, dtype=np.float32)
    b_n = np.asarray(b_n, dtype=np.float32)

    in_maps = []
    for core in range(R * C):
        g, hc = core // C, core % C
        rows = slice(g * BL, (g + 1) * BL)
        hsl = slice(hc * HC, (hc + 1) * HC)
        psl = slice((1 - hc) * HC, (2 - hc) * HC)

        # hx^T with k-tiles my-slice-first; [2048,512]->[128p,16kt,512b]
        hxTf = hx[rows].T
        hxTr = np.concatenate([hxTf[hsl], hxTf[psl]], axis=0)
        hxT = np.ascontiguousarray(
            hxTr.reshape(16, 128, BL).transpose(1, 0, 2).astype(BF))
        inpT = np.ascontiguousarray(
            input[rows].T.reshape(16, 128, BL).transpose(1, 0, 2).astype(BF))

        # w_f rows for my h-slice, k reordered to match hxT; chunk layout
        # [p, c=a*2+half, jj, q] with lhsT(a,j)[p,q] = W[a*128+q, j*128+p]
        Wf = w_f[hsl]
        Wfr = np.concatenate([Wf[:, hsl], Wf[:, psl], Wf[:, H:]], axis=1)
        wf = np.ascontiguousarray(
            Wfr.reshape(NHT, 128, NKT, 128).transpose(3, 0, 2, 1)
            .reshape(128, 16, 16, 128).astype(BF))
        # w_n natural k order (fgx domain is global), [p, a, j, q]
        wn = np.ascontiguousarray(
            w_n[hsl].reshape(NHT, 128, NKT, 128).transpose(3, 0, 2, 1)
            .astype(BF))

        bfp = (b_f[hsl] + 1.0) * 0.5
        in_maps.append({
            "hxT": hxT,
            "inpT": inpT,
            "wf": wf,
            "wn": wn,
            "bfp": np.ascontiguousarray(bfp.reshape(NHT, 128).T, dtype=np.float32),
            "bfm": np.ascontiguousarray((1.0 - bfp).reshape(NHT, 128).T, dtype=np.float32),
            "bn": np.ascontiguousarray(b_n[hsl].reshape(NHT, 128).T, dtype=np.float32),
        })
    return in_maps


def assemble(results):
    """[NHT,128,BL] bf16 hy^T per core -> full [B, H] f32."""
    out = np.empty((B, H), dtype=np.float32)
    for core in range(R * C):
        g, hc = core // C, core % C
        hyT = np.asarray(results[core]["hy"], dtype=np.float32)
        out[g * BL:(g + 1) * BL, hc * HC:(hc + 1) * HC] = \
            hyT.reshape(HC, BL).T
    return out


def kernel(input, hx, w_f, b_f, w_n, b_n, **_ignored):
    nc = _get_nc()
    in_maps = make_in_maps(input, hx, w_f, b_f, w_n, b_n)
    res = run_bass_kernel_spmd(nc, in_maps, list(range(R * C)))
    return assemble(res.results)


if __name__ == "__main__":
    rng = np.random.default_rng(0)
    inputs = {
        "input": rng.uniform(-1, 1, (B, I)).astype(np.float32),
        "hx": rng.uniform(-1, 1, (B, H)).astype(np.float32),
        "w_f": (rng.standard_normal((H, H + I)) / np.sqrt(H + I)).astype(np.float32),
        "b_f": (rng.standard_normal(H) / np.sqrt(H + I)).astype(np.float32),
        "w_n": (rng.standard_normal((H, H + I)) / np.sqrt(H + I)).astype(np.float32),
        "b_n": (rng.standard_normal(H) / np.sqrt(H + I)).astype(np.float32),
    }
    out = kernel(**inputs)
    x64 = {k: v.astype(np.float64) for k, v in inputs.items()}
    cat = np.concatenate([x64["hx"], x64["input"]], axis=1)
    fg = (cat @ x64["w_f"].T + x64["b_f"] + 1.0) * 0.5
    fgx = fg * x64["hx"]
    ng = np.concatenate([fgx, x64["input"]], axis=1) @ x64["w_n"].T + x64["b_n"]
    exp = (1.0 - fg) * ng + fgx
    err = np.abs(out - exp).max() / np.abs(exp).max()
    print("rel err:", err)
